# revision 2
# baseline (speedup 1.0000x reference)
"""Trainium2 Bass kernel for nn_BlockShufflePermuter.

Reference computation (fp32):
    y = x.reshape(-1, 8, 512)                       # [B, c, d]
    cp = sinkhorn(chunk_logits / 0.15)              # [8, 8]
    y = einsum('im,bmd->bid', cp, y)                # chunk mixing
    ip = sinkhorn(intra_logits / 0.15)              # [8, 512, 512]
    y = einsum('bcj,ckj->bck', y, ip)               # per-chunk intra mixing
    out = y.reshape(x.shape)

Device strategy (data-parallel over 8 cores, 2048 tokens each):
  - Load x in "Kron layout": sbuf[(m,bl) partitions, (bh,j) free] via 8
    strided DMAs per 128-token group (2KB contiguous runs in HBM).
  - Fused mix+transpose on the TensorEngine: one fp32 matmul per 128-j
    subtile with stationary lhsT = x-subtile [(m,bl), jr] and moving
    rhs = KRON = CP (x) I_16 [(m,bl),(i,bl)]; psum out = zT[jr, (i,bl)].
  - Round zT to fp32r (tf32) in the PSUM->SBUF copy (DVE).
  - Per-chunk matmul at full PE rate (fp32r, N=512): out[b,k] accumulated
    over 4 j-slices with stationary lhsT = zT-slice, moving rhs = R_i rows.
  - Copy out PSUM->SBUF (ScalarE) and store 2MB contiguous per group.
"""

import numpy as np

TEMPERATURE = 0.15
SINKHORN_ITERS = 5
CHUNKS = 8
DIM = 4096
CHUNK_SIZE = DIM // CHUNKS          # 512
N_CORES = 8
B_TOTAL = 4 * 4096                  # flattened tokens
B_LOCAL = B_TOTAL // N_CORES        # 2048
BG = 128                            # tokens per group (partition dim)
N_GROUPS = B_LOCAL // BG            # 16
NBH = BG // 16                      # 8  (bh index within group)
NS = CHUNK_SIZE // 128              # 4  (j-slices per chunk)

_prog_cache = {}


def _sinkhorn_np(logits: np.ndarray) -> np.ndarray:
    """Float32 Sinkhorn matching the jax reference (row then column lse)."""
    log_p = logits.astype(np.float32)
    for _ in range(SINKHORN_ITERS):
        m = log_p.max(axis=-1, keepdims=True)
        log_p = log_p - (m + np.log(np.sum(np.exp(log_p - m), axis=-1, keepdims=True)))
        m = log_p.max(axis=-2, keepdims=True)
        log_p = log_p - (m + np.log(np.sum(np.exp(log_p - m), axis=-2, keepdims=True)))
    return np.exp(log_p).astype(np.float32)


def _build_program():
    import concourse.bacc as bacc
    import concourse.tile as tile
    import concourse.mybir as mybir

    F32 = mybir.dt.float32
    F32R = mybir.dt.float32r

    nc = bacc.Bacc("TRN2", target_bir_lowering=False, debug=False,
                   num_devices=N_CORES)

    x_d = nc.dram_tensor("x", (B_LOCAL, DIM), F32, kind="ExternalInput").ap()
    kron_d = nc.dram_tensor("kron", (128, 128), F32, kind="ExternalInput").ap()
    # r[jr, c, s, k] = intra_perm[c, k, s*128+jr]
    r_d = nc.dram_tensor("r", (128, CHUNKS * NS * CHUNK_SIZE), F32,
                         kind="ExternalInput").ap()
    o_d = nc.dram_tensor("o", (B_LOCAL, DIM), F32, kind="ExternalOutput").ap()

    with tile.TileContext(nc) as tc:
        with tc.tile_pool(name="const", bufs=1) as const_pool, \
             tc.tile_pool(name="rstage", bufs=2) as rstage, \
             tc.tile_pool(name="xg", bufs=3) as xg_pool, \
             tc.tile_pool(name="zsb", bufs=2) as z_pool, \
             tc.tile_pool(name="osb", bufs=2) as o_pool, \
             tc.tile_pool(name="zps", bufs=3, space="PSUM") as zps, \
             tc.tile_pool(name="ops", bufs=3, space="PSUM") as ops:

            kron_sb = const_pool.tile([128, 128], F32, tag="kron")
            nc.sync.dma_start(kron_sb[:], kron_d)

            # R weights: stage fp32 chunks, round-copy into fp32r residency.
            r_sb = const_pool.tile([128, CHUNKS * NS * CHUNK_SIZE], F32R, tag="r")
            RW = NS * CHUNK_SIZE  # 2048 columns per chunk
            for c in range(CHUNKS):
                stg = rstage.tile([128, RW], F32, tag="rstg")
                nc.sync.dma_start(stg[:], r_d[:, c * RW:(c + 1) * RW])
                nc.vector.tensor_copy(out=r_sb[:, c * RW:(c + 1) * RW], in_=stg[:])

            x_r = x_d.rearrange("(g bh bl) (m j) -> g bh m bl j",
                                bh=NBH, bl=16, m=CHUNKS)

            for g in range(N_GROUPS):
                # ---- load x group in Kron layout: [(m,bl), (bh, j)]
                xg = xg_pool.tile([128, NBH * CHUNK_SIZE], F32, tag="xg")
                for bh in range(NBH):
                    nc.sync.dma_start(
                        xg[:, bh * CHUNK_SIZE:(bh + 1) * CHUNK_SIZE], x_r[g, bh])

                # ---- fused mix+transpose -> zsb[jr, (s, i, bh, bl)]
                zsb = z_pool.tile([128, BG * 32], F32R, tag="zsb")  # 128 x 4096
                zdst = zsb[:].rearrange("p (s i bh bl) -> p s i bh bl",
                                        s=NS, i=CHUNKS, bh=NBH)
                for bh in range(NBH):
                    zp = zps.tile([128, 512], F32)
                    for s in range(NS):
                        nc.tensor.matmul(
                            zp[:, s * 128:(s + 1) * 128],
                            xg[:, bh * CHUNK_SIZE + s * 128: bh * CHUNK_SIZE + (s + 1) * 128],
                            kron_sb[:],
                            start=True, stop=True)
                    nc.vector.tensor_copy(
                        out=zdst[:, :, :, bh, :],
                        in_=zp[:].rearrange("p (s i bl) -> p s i bl", s=NS, i=CHUNKS))

                # ---- per-chunk intra matmul + psum evict + store
                osb = o_pool.tile([128, DIM], F32, tag="osb")
                for i in range(CHUNKS):
                    op = ops.tile([128, CHUNK_SIZE], F32)
                    for s in range(NS):
                        # lhsT: [jr, b=(bh,bl)] contiguous 128; rhs: R_i rows
                        lhsT = zsb[:, (s * CHUNKS + i) * BG:(s * CHUNKS + i + 1) * BG]
                        rhs = r_sb[:, i * RW + s * CHUNK_SIZE: i * RW + (s + 1) * CHUNK_SIZE]
                        nc.tensor.matmul(op[:], lhsT, rhs,
                                         start=(s == 0), stop=(s == NS - 1))
                    nc.scalar.copy(
                        out=osb[:, i * CHUNK_SIZE:(i + 1) * CHUNK_SIZE], in_=op[:])

                nc.sync.dma_start(o_d[g * BG:(g + 1) * BG, :], osb[:])

    nc.compile()
    return nc


def kernel(x: np.ndarray, chunk_logits: np.ndarray, intra_logits: np.ndarray) -> np.ndarray:
    from concourse.bass_utils import run_bass_kernel_spmd

    orig_shape = x.shape
    orig_dtype = x.dtype

    cp = _sinkhorn_np(np.asarray(chunk_logits, dtype=np.float32) / TEMPERATURE)
    ip = _sinkhorn_np(np.asarray(intra_logits, dtype=np.float32) / TEMPERATURE)

    # KRON[(m,bl'), (i,bl)] = cp[i, m] * (bl' == bl)
    kron = np.zeros((128, 128), dtype=np.float32)
    for m in range(CHUNKS):
        for i in range(CHUNKS):
            for bl in range(16):
                kron[m * 16 + bl, i * 16 + bl] = cp[i, m]

    # r[jr, c, s, k] = ip[c, k, s*128+jr]
    r = ip.transpose(2, 0, 1)                       # [j, c, k]
    r = r.reshape(NS, 128, CHUNKS, CHUNK_SIZE)      # [s, jr, c, k]
    r = r.transpose(1, 2, 0, 3)                     # [jr, c, s, k]
    r = np.ascontiguousarray(r).reshape(128, CHUNKS * NS * CHUNK_SIZE)

    xf = np.ascontiguousarray(np.asarray(x, dtype=np.float32).reshape(B_TOTAL, DIM))

    if "prog" not in _prog_cache:
        _prog_cache["prog"] = _build_program()
    nc = _prog_cache["prog"]

    in_maps = [
        {"x": xf[c * B_LOCAL:(c + 1) * B_LOCAL], "kron": kron, "r": r}
        for c in range(N_CORES)
    ]
    res = run_bass_kernel_spmd(nc, in_maps, core_ids=list(range(N_CORES)))
    out = np.concatenate([res.results[c]["o"] for c in range(N_CORES)], axis=0)
    return out.reshape(orig_shape).astype(orig_dtype, copy=False)


# revision 5
# speedup vs baseline: 8.6869x; 8.6869x over previous
"""Trainium2 Bass kernel for nn_BlockShufflePermuter.

Reference computation (fp32):
    y = x.reshape(-1, 8, 512)                       # [B, c, d]
    cp = sinkhorn(chunk_logits / 0.15)              # [8, 8]
    y = einsum('im,bmd->bid', cp, y)                # chunk mixing
    ip = sinkhorn(intra_logits / 0.15)              # [8, 512, 512]
    y = einsum('bcj,ckj->bck', y, ip)               # per-chunk intra mixing
    out = y.reshape(x.shape)

Device strategy (data-parallel over 8 cores, 2048 tokens each):
  - Load x in "Kron layout": sbuf[(m,bl) partitions, (bh,j) free] via 8
    strided DMAs per 128-token group (2KB contiguous runs in HBM).
  - Fused mix+transpose on the TensorEngine: one fp32 matmul per 128-j
    subtile with stationary lhsT = x-subtile [(m,bl), jr] and moving
    rhs = KRON = CP (x) I_16 [(m,bl),(i,bl)]; psum out = zT[jr, (i,bl)].
  - Round zT to fp32r (tf32) in the PSUM->SBUF copy (DVE), rearranged so
    each (s, i) slice has its 128 b-columns contiguous.
  - Per-chunk matmul at full PE rate (fp32r, N=512): out[b,k] accumulated
    over 4 j-slices with stationary lhsT = zT-slice, moving rhs = R_i rows.
  - Copy out PSUM->SBUF (ScalarE) and store 2MB contiguous per group.
"""

import numpy as np

TEMPERATURE = 0.15
SINKHORN_ITERS = 5
CHUNKS = 8
DIM = 4096
CHUNK_SIZE = DIM // CHUNKS          # 512
N_CORES = 8
B_TOTAL = 4 * 4096                  # flattened tokens
B_LOCAL = B_TOTAL // N_CORES        # 2048
BG = 128                            # tokens per group (partition dim)
N_GROUPS = B_LOCAL // BG            # 16
NBH = BG // 16                      # 8  (bh index within group)
NS = CHUNK_SIZE // 128              # 4  (j-slices per chunk)
RW = NS * CHUNK_SIZE                # 2048 R columns per chunk

_prog_cache = {}


def _sinkhorn_np(logits: np.ndarray) -> np.ndarray:
    """Float32 Sinkhorn matching the jax reference (row then column lse)."""
    log_p = logits.astype(np.float32)
    for _ in range(SINKHORN_ITERS):
        m = log_p.max(axis=-1, keepdims=True)
        log_p = log_p - (m + np.log(np.sum(np.exp(log_p - m), axis=-1, keepdims=True)))
        m = log_p.max(axis=-2, keepdims=True)
        log_p = log_p - (m + np.log(np.sum(np.exp(log_p - m), axis=-2, keepdims=True)))
    return np.exp(log_p).astype(np.float32)


def make_weights(chunk_logits: np.ndarray, intra_logits: np.ndarray):
    """Host-side constants: KRON (CP (x) I_16) and R (intra perms, j-major)."""
    cp = _sinkhorn_np(np.asarray(chunk_logits, dtype=np.float32) / TEMPERATURE)
    ip = _sinkhorn_np(np.asarray(intra_logits, dtype=np.float32) / TEMPERATURE)

    kron = np.zeros((128, 128), dtype=np.float32)
    idx = np.arange(16)
    for m in range(CHUNKS):
        for i in range(CHUNKS):
            kron[m * 16 + idx, i * 16 + idx] = cp[i, m]

    # r[jr, c, s, k] = ip[c, k, s*128+jr]
    r = ip.transpose(2, 0, 1)                       # [j, c, k]
    r = r.reshape(NS, 128, CHUNKS, CHUNK_SIZE)      # [s, jr, c, k]
    r = np.ascontiguousarray(r.transpose(1, 2, 0, 3)).reshape(128, CHUNKS * RW)
    return kron, r


def _emit_body(nc, tc, mybir, x_r, o_d, kron_sb, r_sb, pools):
    F32 = mybir.dt.float32
    F32R = mybir.dt.float32r
    xg_pool, z_pool, o_pool, zps, ops = pools

    for g in range(N_GROUPS):
        # ---- load x group in Kron layout: [(m,bl), (bh, j)]
        xg = xg_pool.tile([128, NBH * CHUNK_SIZE], F32, tag="xg")
        for bh in range(NBH):
            nc.sync.dma_start(
                xg[:, bh * CHUNK_SIZE:(bh + 1) * CHUNK_SIZE], x_r[g, bh])

        # ---- fused mix+transpose -> zsb[jr, (s, i, bh, bl)]
        zsb = z_pool.tile([128, BG * 32], F32R, tag="zsb")  # 128 x 4096
        zdst = zsb[:].rearrange("p (s i bh bl) -> p s i bh bl",
                                s=NS, i=CHUNKS, bh=NBH)
        for bh in range(NBH):
            zp = zps.tile([128, 512], F32)
            for s in range(NS):
                nc.tensor.matmul(
                    zp[:, s * 128:(s + 1) * 128],
                    xg[:, bh * CHUNK_SIZE + s * 128: bh * CHUNK_SIZE + (s + 1) * 128],
                    kron_sb[:],
                    start=True, stop=True)
            nc.vector.tensor_copy(
                out=zdst[:, :, :, bh, :],
                in_=zp[:].rearrange("p (s i bl) -> p s i bl", s=NS, i=CHUNKS))

        # ---- per-chunk intra matmul + psum evict + store
        osb = o_pool.tile([128, DIM], F32, tag="osb")
        for i in range(CHUNKS):
            op = ops.tile([128, CHUNK_SIZE], F32)
            for s in range(NS):
                # lhsT: [jr, b=(bh,bl)] contiguous 128; rhs: R_i rows
                lhsT = zsb[:, (s * CHUNKS + i) * BG:(s * CHUNKS + i + 1) * BG]
                rhs = r_sb[:, i * RW + s * CHUNK_SIZE: i * RW + (s + 1) * CHUNK_SIZE]
                nc.tensor.matmul(op[:], lhsT, rhs,
                                 start=(s == 0), stop=(s == NS - 1))
            nc.scalar.copy(
                out=osb[:, i * CHUNK_SIZE:(i + 1) * CHUNK_SIZE], in_=op[:])

        nc.sync.dma_start(o_d[g * BG:(g + 1) * BG, :], osb[:])


def _build_program(repeats: int = 1):
    """Build the per-core program. repeats>1 wraps the body in a hardware
    For_i loop (used only for timing measurement)."""
    import contextlib
    import concourse.bacc as bacc
    import concourse.tile as tile
    import concourse.mybir as mybir

    F32 = mybir.dt.float32
    F32R = mybir.dt.float32r

    nc = bacc.Bacc("TRN2", target_bir_lowering=False, debug=False,
                   num_devices=N_CORES)

    x_d = nc.dram_tensor("x", (B_LOCAL, DIM), F32, kind="ExternalInput").ap()
    kron_d = nc.dram_tensor("kron", (128, 128), F32, kind="ExternalInput").ap()
    # r[jr, c, s, k] = intra_perm[c, k, s*128+jr]
    r_d = nc.dram_tensor("r", (128, CHUNKS * RW), F32, kind="ExternalInput").ap()
    o_d = nc.dram_tensor("o", (B_LOCAL, DIM), F32, kind="ExternalOutput").ap()

    with tile.TileContext(nc) as tc:
        with tc.tile_pool(name="const", bufs=1) as const_pool, \
             tc.tile_pool(name="rstage", bufs=2) as rstage, \
             tc.tile_pool(name="xg", bufs=3) as xg_pool, \
             tc.tile_pool(name="zsb", bufs=2) as z_pool, \
             tc.tile_pool(name="osb", bufs=2) as o_pool, \
             tc.tile_pool(name="zps", bufs=3, space="PSUM") as zps, \
             tc.tile_pool(name="ops", bufs=3, space="PSUM") as ops:

            kron_sb = const_pool.tile([128, 128], F32, tag="kron")
            nc.sync.dma_start(kron_sb[:], kron_d)

            # R weights: stage fp32 chunks, round-copy into fp32r residency.
            r_sb = const_pool.tile([128, CHUNKS * RW], F32R, tag="r")
            for c in range(CHUNKS):
                stg = rstage.tile([128, RW], F32, tag="rstg")
                nc.sync.dma_start(stg[:], r_d[:, c * RW:(c + 1) * RW])
                nc.vector.tensor_copy(out=r_sb[:, c * RW:(c + 1) * RW], in_=stg[:])

            x_r = x_d.rearrange("(g bh bl) (m j) -> g bh m bl j",
                                bh=NBH, bl=16, m=CHUNKS)

            pools = (xg_pool, z_pool, o_pool, zps, ops)
            if repeats > 1:
                with tc.For_i(0, repeats, 1):
                    _emit_body(nc, tc, mybir, x_r, o_d, kron_sb, r_sb, pools)
            else:
                _emit_body(nc, tc, mybir, x_r, o_d, kron_sb, r_sb, pools)

    nc.compile()
    return nc


def kernel(x: np.ndarray, chunk_logits: np.ndarray, intra_logits: np.ndarray) -> np.ndarray:
    from concourse.bass_utils import run_bass_kernel_spmd

    orig_shape = x.shape
    orig_dtype = x.dtype

    kron, r = make_weights(chunk_logits, intra_logits)
    xf = np.ascontiguousarray(np.asarray(x, dtype=np.float32).reshape(B_TOTAL, DIM))

    if "prog" not in _prog_cache:
        _prog_cache["prog"] = _build_program()
    nc = _prog_cache["prog"]

    in_maps = [
        {"x": xf[c * B_LOCAL:(c + 1) * B_LOCAL], "kron": kron, "r": r}
        for c in range(N_CORES)
    ]
    res = run_bass_kernel_spmd(nc, in_maps, core_ids=list(range(N_CORES)))
    out = np.concatenate([res.results[c]["o"] for c in range(N_CORES)], axis=0)
    return out.reshape(orig_shape).astype(orig_dtype, copy=False)
